# revision 1
# baseline (speedup 1.0000x reference)
"""Kuramoto oscillator network kernel for 8 Trainium2 NeuronCores.

Problem: B=256 batches, D=256 feature dims, N=16 oscillator dims, T=25 steps.
    c = emb[:,:,None]*W_d + b_d                        [B,D,N]
    x = normalize(noise + c)                            (init, per (b,d) over N)
    repeat T: f1 = J_in@x1 + J_out@x2 + c1  (einsum ijkl,bjl->bik)
              p  = f - <x,f>x ; om = Omega@x
              x  = normalize(x + g*(om + p))
    out = stack(x1, x2)                                 [2,B,D,N]

Strategy (model-parallel over output dim i, all-transposed layout):
  * Each core owns a 512-wide slice of the flattened ik axis (32 of 256 i's).
    J_in^T / J_out^T slices ([4096 x 512] each) stay resident in SBUF and are
    used as matmul stationary tiles in float32r (FP22 read truncation, full
    bf16-rate on the PE vs 4x slower true fp32).
  * State X^T [(j,l), batchcol] lives in HBM, AllGather'd across cores each
    step; columns are [x1|x2] so the J_out cross-coupling is a column-swapped
    rhs view (negative-stride AP).
  * Omega rotation is 4 extra block-diagonal matmuls accumulated into the same
    PSUM as f (skew-symmetry makes <x,Omega x>=0, so the tangent projection is
    unaffected).
  * Per-(b,i) reductions over the 16 oscillator partitions (projection <x,f>
    and the normalize norm) are single matmuls against a block-ones matrix
    that reduce AND broadcast in one shot.
  * Batches split into two groups (A=0:128, B=128:256) pipelined so each
    group's AllGather hides under the other group's matmul phase.

Self-contained: hardcodes shapes; no imports from /root/problem.
"""

import os
import sys
import time

sys.path.insert(0, "/opt/trn_rl_repo")

import numpy as np

import concourse.bass as bass
import concourse.mybir as mybir
import concourse.tile as tile
from concourse import bacc
from concourse import bass2jax
from concourse.bass_interp import get_hw_module

B, D, N = 256, 256, 16
DN = D * N                      # 4096 flattened (i,k) / (j,l)
T = int(os.environ.get("KUR_T", "25"))
GAMMA = 0.1
NCORES = 8
IKS = DN // NCORES              # 512 ik per core (32 i values)
NIPC = D // NCORES              # 32 i per core
GCOL = 256                      # columns per batch group (128 x1 + 128 x2)
HALF = 128

FP32 = mybir.dt.float32
FP32R = mybir.dt.float32r
FP16 = mybir.dt.float16

_CACHE = {}


def _swap_halves(ap):
    """View a [128, 256] SBUF AP with its two 128-column halves swapped."""
    return bass.AP(
        tensor=ap.tensor,
        offset=ap.offset + HALF,
        ap=[list(ap.ap[0])] + [[-HALF, 2], [1, HALF]],
    )


def _build(nc):
    AF = mybir.ActivationFunctionType

    # ---------------- DRAM I/O ----------------
    jt_in_d = nc.dram_tensor("jt_in", [DN, IKS], FP16, kind="ExternalInput")
    jt_out_d = nc.dram_tensor("jt_out", [DN, IKS], FP16, kind="ExternalInput")
    ombd_d = nc.dram_tensor("ombd", [4 * HALF, HALF], FP32R, kind="ExternalInput")
    wdiag_d = nc.dram_tensor("wdiag", [32 * HALF, HALF], FP32, kind="ExternalInput")
    wdiag_s_d = nc.dram_tensor("wdiag_s", [4 * HALF, HALF], FP32, kind="ExternalInput")
    bones_d = nc.dram_tensor("bones", [HALF, HALF], FP32, kind="ExternalInput")
    embt_d = nc.dram_tensor("embt", [2 * HALF, 2 * GCOL], FP32, kind="ExternalInput")
    embt_own_d = nc.dram_tensor("embt_own", [HALF, 2 * GCOL], FP32, kind="ExternalInput")
    bdt_d = nc.dram_tensor("bdt", [HALF, 32], FP32, kind="ExternalInput")
    bdt_s_d = nc.dram_tensor("bdt_s", [HALF, 4], FP32, kind="ExternalInput")
    noiset_d = nc.dram_tensor("noiset", [DN, 2 * GCOL], FP32, kind="ExternalInput")
    noiset_own_d = nc.dram_tensor("noiset_own", [IKS, 2 * GCOL], FP32, kind="ExternalInput")

    xt_out_d = nc.dram_tensor("xt_out", [IKS, 2 * GCOL], FP32, kind="ExternalOutput")
    DBG = os.environ.get("KUR_DBG", "0") == "1"
    if DBG:
        dbg_h_d = nc.dram_tensor("dbg_h", [HALF, GCOL], FP32, kind="ExternalOutput")

    # internal HBM: gathered state + AG input bounce, per group
    xg_t = [
        nc.dram_tensor(f"xg{g}_t", [DN, GCOL], FP16, addr_space="Shared")
        for g in range(2)
    ]
    agin = [nc.dram_tensor(f"agin{g}", [IKS, GCOL], FP16) for g in range(2)]

    with tile.TileContext(nc) as tc:
        with (
            tc.tile_pool(name="res", bufs=1) as res,
            tc.tile_pool(name="stream", bufs=4) as stream,
            tc.tile_pool(name="xstream", bufs=8) as xstream,
            tc.tile_pool(name="tmp", bufs=2) as tmp,
            tc.tile_pool(name="fps", bufs=2, space="PSUM") as fps,
            tc.tile_pool(name="auxps", bufs=3, space="PSUM") as auxps,
            tc.tile_pool(name="dram", bufs=1, space="DRAM") as _dr,
        ):
            # ---------------- resident SBUF ----------------
            j_in_sb = res.tile([HALF, 32 * IKS], FP16, tag="jin")
            j_out_sb = res.tile([HALF, 32 * IKS], FP16, tag="jout")
            for k in range(32):
                nc.sync.dma_start(
                    out=j_in_sb[:, k * IKS:(k + 1) * IKS],
                    in_=jt_in_d[k * HALF:(k + 1) * HALF, :],
                )
                nc.sync.dma_start(
                    out=j_out_sb[:, k * IKS:(k + 1) * IKS],
                    in_=jt_out_d[k * HALF:(k + 1) * HALF, :],
                )

            ombd_sb = res.tile([HALF, 4 * HALF], FP32R, tag="ombd")
            for m in range(4):
                nc.sync.dma_start(
                    out=ombd_sb[:, m * HALF:(m + 1) * HALF],
                    in_=ombd_d[m * HALF:(m + 1) * HALF, :],
                )
            bones_r = res.tile([HALF, HALF], FP32R, tag="bones_r")
            bones_f = res.tile([HALF, HALF], FP32, tag="bones_f")
            nc.sync.dma_start(out=bones_r[:, :], in_=bones_d[:, :].bitcast(FP32R))
            nc.sync.dma_start(out=bones_f[:, :], in_=bones_d[:, :])

            embt_sb = res.tile([HALF, 2 * 2 * GCOL], FP32, tag="embt")
            for r in range(2):
                nc.sync.dma_start(
                    out=embt_sb[:, r * 2 * GCOL:(r + 1) * 2 * GCOL],
                    in_=embt_d[r * HALF:(r + 1) * HALF, :],
                )
            embt_own_sb = res.tile([HALF, 2 * GCOL], FP32, tag="embt_own")
            nc.sync.dma_start(out=embt_own_sb[:, :], in_=embt_own_d[:, :])
            bdt_sb = res.tile([HALF, 32], FP32, tag="bdt")
            nc.sync.dma_start(out=bdt_sb[:, :], in_=bdt_d[:, :])
            bdt_s_sb = res.tile([HALF, 4], FP32, tag="bdt_s")
            nc.sync.dma_start(out=bdt_s_sb[:, :], in_=bdt_s_d[:, :])

            # state slices (own ik rows), fp32 + rounded fp32r copy
            xs = res.tile([HALF, 8 * GCOL], FP32, tag="xs")
            xsr = res.tile([HALF, 8 * GCOL], FP32R, tag="xsr")
            cs = res.tile([HALF, 8 * GCOL], FP32, tag="cs")

            # ---------------- init: full X0 -> xg_t ----------------
            def normalize_into(u2, bones_tile, dt_rhs, out_sl_writer):
                """u2: [128,256] fp32 SBUF tile. Writes normalized result."""
                sq = tmp.tile([HALF, GCOL], dt_rhs, tag="sq")
                nc.vector.tensor_mul(out=sq[:, :], in0=u2[:, :], in1=u2[:, :])
                nb = auxps.tile([HALF, GCOL], FP32, tag="aux")
                nc.tensor.matmul(
                    nb[:, :], bones_tile[:, :], sq[:, :], start=True, stop=True
                )
                nrm = tmp.tile([HALF, GCOL], FP32, tag="nrm")
                nc.scalar.activation(nrm[:, :], nb[:, :], AF.Sqrt)
                rinv = tmp.tile([HALF, GCOL], FP32, tag="rinv")
                nc.vector.reciprocal(out=rinv[:, :], in_=nrm[:, :])
                out_sl_writer(u2, rinv)

            for g in range(2):
                for kk in range(32):
                    cps = auxps.tile([HALF, GCOL], FP32, tag="aux")
                    wt = stream.tile([HALF, HALF], FP32, tag="wdiag")
                    nc.sync.dma_start(
                        out=wt[:, :], in_=wdiag_d[kk * HALF:(kk + 1) * HALF, :]
                    )
                    nc.tensor.matmul(
                        cps[:, :],
                        wt[:, :],
                        embt_sb[:, (kk // 16) * 2 * GCOL + g * GCOL:][:, :GCOL],
                        start=True,
                        stop=True,
                    )
                    nt = stream.tile([HALF, GCOL], FP32, tag="noise")
                    nc.sync.dma_start(
                        out=nt[:, :],
                        in_=noiset_d[kk * HALF:(kk + 1) * HALF, g * GCOL:(g + 1) * GCOL],
                    )
                    u2 = tmp.tile([HALF, GCOL], FP32, tag="u2")
                    # u2 = (cps + b_d_col) + noise
                    nc.vector.scalar_tensor_tensor(
                        out=u2[:, :],
                        in0=cps[:, :],
                        scalar=bdt_sb[:, kk:kk + 1],
                        in1=nt[:, :],
                        op0=mybir.AluOpType.add,
                        op1=mybir.AluOpType.add,
                    )

                    def wr(u2_, rinv_, g=g, kk=kk):
                        x0 = tmp.tile([HALF, GCOL], FP16, tag="x0")
                        nc.vector.tensor_mul(out=x0[:, :], in0=u2_[:, :], in1=rinv_[:, :])
                        nc.sync.dma_start(
                            out=xg_t[g][kk * HALF:(kk + 1) * HALF, :], in_=x0[:, :]
                        )

                    normalize_into(u2, bones_f, FP32, wr)

            # ---------------- init: own-slice c_s and x_s ----------------
            for g in range(2):
                for m in range(4):
                    sl = slice((g * 4 + m) * GCOL, (g * 4 + m + 1) * GCOL)
                    cps = auxps.tile([HALF, GCOL], FP32, tag="aux")
                    wt = stream.tile([HALF, HALF], FP32, tag="wdiag")
                    nc.sync.dma_start(
                        out=wt[:, :], in_=wdiag_s_d[m * HALF:(m + 1) * HALF, :]
                    )
                    nc.tensor.matmul(
                        cps[:, :],
                        wt[:, :],
                        embt_own_sb[:, g * GCOL:(g + 1) * GCOL],
                        start=True,
                        stop=True,
                    )
                    nc.vector.tensor_scalar_add(
                        cs[:, sl], cps[:, :], bdt_s_sb[:, m:m + 1]
                    )
                    nt = stream.tile([HALF, GCOL], FP32, tag="noise")
                    nc.sync.dma_start(
                        out=nt[:, :],
                        in_=noiset_own_d[m * HALF:(m + 1) * HALF, g * GCOL:(g + 1) * GCOL],
                    )
                    u2 = tmp.tile([HALF, GCOL], FP32, tag="u2")
                    nc.vector.tensor_add(out=u2[:, :], in0=cs[:, sl], in1=nt[:, :])

                    def wr(u2_, rinv_, sl=sl):
                        nc.vector.tensor_mul(out=xs[:, sl], in0=u2_[:, :], in1=rinv_[:, :])
                        nc.vector.tensor_copy(out=xsr[:, sl], in_=xs[:, sl])

                    normalize_into(u2, bones_f, FP32, wr)

            # ---------------- main loop ----------------
            prev_cc = [None, None]
            for t in range(T):
                for g in range(2):
                    f01 = fps.tile([HALF, 2 * GCOL], FP32, tag=f"f{g}")
                    f23 = fps.tile([HALF, 2 * GCOL], FP32, tag=f"f{g}")

                    def freg(m):
                        ft = f01 if m < 2 else f23
                        c0 = (m % 2) * GCOL
                        return ft[:, c0:c0 + GCOL]

                    for k in range(32):
                        xk = xstream.tile([HALF, GCOL], FP16, tag=f"xg{g}")
                        dma = nc.sync.dma_start(
                            out=xk[:, :],
                            in_=xg_t[g][k * HALF:(k + 1) * HALF, :],
                        )
                        if prev_cc[g] is not None:
                            tile.add_dep_helper(
                                dma.ins, prev_cc[g].ins, reason="AG->stream RAW"
                            )
                        xk_sw = _swap_halves(xk[:, :])
                        for m in range(4):
                            # start=True clears the whole PSUM bank's
                            # has_written bits -- issue it only on the first
                            # matmul ever touching each bank (m=0/m=2, k=0).
                            nc.tensor.matmul(
                                freg(m),
                                j_in_sb[:, k * IKS + m * HALF:][:, :HALF],
                                xk[:, :],
                                start=(k == 0 and m % 2 == 0),
                                stop=False,
                                skip_group_check=True,
                            )
                            nc.tensor.matmul(
                                freg(m),
                                j_out_sb[:, k * IKS + m * HALF:][:, :HALF],
                                xk_sw,
                                start=False,
                                stop=False,
                                skip_group_check=True,
                            )

                    for m in range(4):
                        sl = slice((g * 4 + m) * GCOL, (g * 4 + m + 1) * GCOL)
                        # Omega rotation into same accumulation
                        nc.tensor.matmul(
                            freg(m),
                            ombd_sb[:, m * HALF:(m + 1) * HALF],
                            xsr[:, sl],
                            start=False,
                            stop=(m % 2 == 1),
                            skip_group_check=True,
                        )
                        h = tmp.tile([HALF, GCOL], FP32, tag="h")
                        nc.vector.tensor_add(out=h[:, :], in0=freg(m), in1=cs[:, sl])
                        if DBG and t == 0 and g == 0 and m == 0:
                            nc.sync.dma_start(out=dbg_h_d[:, :], in_=h[:, :])
                        xf = tmp.tile([HALF, GCOL], FP32R, tag="xf")
                        nc.vector.tensor_mul(out=xf[:, :], in0=h[:, :], in1=xs[:, sl])
                        sb_ps = auxps.tile([HALF, GCOL], FP32, tag="aux")
                        nc.tensor.matmul(
                            sb_ps[:, :], bones_r[:, :], xf[:, :], start=True, stop=True
                        )
                        t1 = tmp.tile([HALF, GCOL], FP32, tag="t1")
                        nc.vector.scalar_tensor_tensor(
                            out=t1[:, :],
                            in0=sb_ps[:, :],
                            scalar=GAMMA,
                            in1=xs[:, sl],
                            op0=mybir.AluOpType.mult,
                            op1=mybir.AluOpType.mult,
                        )
                        t2 = tmp.tile([HALF, GCOL], FP32, tag="t2")
                        nc.vector.scalar_tensor_tensor(
                            out=t2[:, :],
                            in0=h[:, :],
                            scalar=GAMMA,
                            in1=xs[:, sl],
                            op0=mybir.AluOpType.mult,
                            op1=mybir.AluOpType.add,
                        )
                        pre = tmp.tile([HALF, GCOL], FP32, tag="pre")
                        nc.vector.tensor_sub(out=pre[:, :], in0=t2[:, :], in1=t1[:, :])

                        def wr(pre_, rinv_, sl=sl, m=m, g=g, t=t):
                            nc.vector.tensor_mul(
                                out=xs[:, sl], in0=pre_[:, :], in1=rinv_[:, :]
                            )
                            nc.vector.tensor_copy(out=xsr[:, sl], in_=xs[:, sl])
                            if t < T - 1:
                                xh = tmp.tile([HALF, GCOL], FP16, tag="xh")
                                nc.vector.tensor_copy(out=xh[:, :], in_=xs[:, sl])
                                nc.sync.dma_start(
                                    out=agin[g][m * HALF:(m + 1) * HALF, :], in_=xh[:, :]
                                )

                        normalize_into(pre, bones_r, FP32R, wr)

                    if t < T - 1:
                        cc = nc.gpsimd.collective_compute(
                            "AllGather",
                            mybir.AluOpType.bypass,
                            replica_groups=[list(range(NCORES))],
                            ins=[agin[g][:, :].opt()],
                            outs=[xg_t[g][:, :].opt()],
                        )
                        prev_cc[g] = cc

            # ---------------- output: own ik rows, fp32 ----------------
            for g in range(2):
                for m in range(4):
                    sl = slice((g * 4 + m) * GCOL, (g * 4 + m + 1) * GCOL)
                    nc.sync.dma_start(
                        out=xt_out_d[m * HALF:(m + 1) * HALF, g * GCOL:(g + 1) * GCOL],
                        in_=xs[:, sl],
                    )

    nc.compile()
    return nc


def _get_nc():
    if "nc" not in _CACHE:
        nc = bacc.Bacc(
            "TRN2", target_bir_lowering=False, debug=False, num_devices=NCORES
        )
        _build(nc)
        nc.m = get_hw_module(nc.m)
        _CACHE["nc"] = nc
    return _CACHE["nc"]


def _marshal(embeddings1, embeddings2, W_d, b_d, J_in, J_out, Omega, noise1, noise2):
    """Host-side pure data movement: slice/transpose/scatter into device layout."""
    f32 = np.float32

    # J^T[(j,l),(i,k)] = J[i,j,k,l]
    jt_in = np.ascontiguousarray(J_in.transpose(1, 3, 0, 2).reshape(DN, DN))
    jt_out = np.ascontiguousarray(J_out.transpose(1, 3, 0, 2).reshape(DN, DN))

    # column layout: [x1_A | x2_A | x1_B | x2_B], 128 batches each
    def colcat(a1, a2):  # a1,a2: [B, ...] -> stacked columns
        return np.concatenate(
            [a1[:HALF], a2[:HALF], a1[HALF:], a2[HALF:]], axis=0
        )

    embt = np.ascontiguousarray(colcat(embeddings1, embeddings2).T)      # [256,512]
    nt1 = noise1.transpose(1, 2, 0).reshape(DN, B)                       # [(d,n), b]
    nt2 = noise2.transpose(1, 2, 0).reshape(DN, B)
    noiset = np.ascontiguousarray(
        np.concatenate([nt1[:, :HALF], nt2[:, :HALF], nt1[:, HALF:], nt2[:, HALF:]], axis=1)
    )

    bdt_flat = np.ascontiguousarray(b_d.reshape(DN))
    bdt = np.ascontiguousarray(bdt_flat.reshape(32, HALF).T)             # [128, 32]

    # W_diag scatter: block kk covers i in [8kk, 8kk+8)
    wdiag = np.zeros((32, HALF, HALF), f32)
    for kk in range(32):
        base_row = 128 * (kk // 16)
        for iloc in range(8):
            i = 8 * kk + iloc
            wdiag[kk, i - base_row, iloc * 16:(iloc + 1) * 16] = W_d[i]
    wdiag = wdiag.reshape(32 * HALF, HALF)

    bones = np.kron(np.eye(8, dtype=f32), np.ones((16, 16), f32))

    in_maps = []
    for c in range(NCORES):
        i0 = NIPC * c
        ik0 = IKS * c
        ombd = np.zeros((4, HALF, HALF), f32)
        for m in range(4):
            for iloc in range(8):
                i = i0 + 8 * m + iloc
                ombd[m, iloc * 16:(iloc + 1) * 16, iloc * 16:(iloc + 1) * 16] = Omega[i].T
        in_maps.append(
            {
                "jt_in": np.ascontiguousarray(jt_in[:, ik0:ik0 + IKS]).astype(np.float16),
                "jt_out": np.ascontiguousarray(jt_out[:, ik0:ik0 + IKS]).astype(np.float16),
                "ombd": ombd.reshape(4 * HALF, HALF),
                "wdiag": wdiag,
                "wdiag_s": np.ascontiguousarray(
                    wdiag.reshape(32, HALF, HALF)[4 * c:4 * c + 4].reshape(4 * HALF, HALF)
                ),
                "bones": bones,
                "embt": embt,
                "embt_own": np.ascontiguousarray(
                    embt[128 * (c // 4):128 * (c // 4) + 128]
                ),
                "bdt": bdt,
                "bdt_s": np.ascontiguousarray(
                    bdt_flat[ik0:ik0 + IKS].reshape(4, HALF).T
                ),
                "noiset": noiset,
                "noiset_own": np.ascontiguousarray(noiset[ik0:ik0 + IKS]),
            }
        )
    return in_maps


def _unmarshal(xt):
    """xt: [4096, 512] -> [2, B, D, N]"""
    x1 = np.concatenate([xt[:, 0:HALF], xt[:, 2 * HALF:3 * HALF]], axis=1)
    x2 = np.concatenate([xt[:, HALF:2 * HALF], xt[:, 3 * HALF:4 * HALF]], axis=1)
    out = np.empty((2, B, D, N), np.float32)
    out[0] = x1.reshape(D, N, B).transpose(2, 0, 1)
    out[1] = x2.reshape(D, N, B).transpose(2, 0, 1)
    return out


def run_on_device(in_maps):
    nc = _get_nc()
    return bass2jax.run_bass_via_pjrt(nc, in_maps, n_cores=NCORES)


def kernel(**inputs):
    in_maps = _marshal(**{k: np.asarray(v, np.float32) for k, v in inputs.items()})
    results = run_on_device(in_maps)
    xt = np.concatenate([results[c]["xt_out"] for c in range(NCORES)], axis=0)
    return _unmarshal(xt)


if __name__ == "__main__":
    rng = np.random.default_rng(0)
    ins = {
        "embeddings1": rng.standard_normal((B, D), dtype=np.float32),
        "embeddings2": rng.standard_normal((B, D), dtype=np.float32),
        "W_d": rng.standard_normal((D, N), dtype=np.float32) * 0.1,
        "b_d": np.zeros((D, N), np.float32),
        "J_in": (rng.standard_normal((D, D, N, N), dtype=np.float32) * 0.007),
        "J_out": (rng.standard_normal((D, D, N, N), dtype=np.float32) * 0.007),
        "Omega": rng.standard_normal((D, N, N), dtype=np.float32) * 0.1,
        "noise1": rng.standard_normal((B, D, N), dtype=np.float32) * 0.05,
        "noise2": rng.standard_normal((B, D, N), dtype=np.float32) * 0.05,
    }
    t0 = time.time()
    out = kernel(**ins)
    print("kernel() took", time.time() - t0, "s; out shape", out.shape)



# revision 11
# speedup vs baseline: 1.1996x; 1.1996x over previous
"""Kuramoto oscillator network kernel for 8 Trainium2 NeuronCores.

Problem: B=256 batches, D=256 feature dims, N=16 oscillator dims, T=25 steps.
    c = emb[:,:,None]*W_d + b_d                        [B,D,N]
    x = normalize(noise + c)                            (init, per (b,d) over N)
    repeat T: f1 = J_in@x1 + J_out@x2 + c1  (einsum ijkl,bjl->bik)
              p  = f - <x,f>x ; om = Omega@x
              x  = normalize(x + g*(om + p))
    out = stack(x1, x2)                                 [2,B,D,N]

Strategy (v2): sum/difference symmetrization + transposed matmul geometry.
  * With s=x1+x2, d=x1-x2, A=(J_in+J_out)/2, Bm=(J_in-J_out)/2:
        f1 = A s + Bm d,  f2 = A s - Bm d
    -- HALF the matmul FLOPs of the naive 4-einsum form.
  * Omega rotation is skew-symmetric (<x,Omega x>=0) so it can be folded
    into A and Bm on the host (block-diagonal add) -- zero device cost.
  * The conditional stimulus c enters f every step; (c1+c2)/2 and
    (c1-c2)/2 are rank-structured (emb (x) W_d) and are folded into the
    same PSUM accumulation as 2 extra matmul chunks (emb^T stationary,
    block-diag W moving).
  * Transposed geometry: stationary = gathered state chunk [jl=128, b=128],
    moving = A/Bm column slice [128, 512] (fp16, N=512 per matmul).
    Output lands batch-major [b, ik], which makes the per-(b,i) reductions
    (projection <x,f> and the normalize norm) native windowed DVE reduces
    (shape [128, 32, 16] axis-X), eliminating the block-ones matmuls and
    the giant broadcast reciprocal of v1.
  * Model-parallel over ik (each core owns 512 of 4096 ik), batch kept
    whole per matmul; batches split in 2 groups only for AllGather/compute
    pipelining. Per step per group: 64 matmuls N=512 + 3 c-fold matmuls +
    8 PE transposes (to return x' to [jl, b] layout for the AllGather).
  * Elementwise work is split across Vector (reduces + state-1 chain),
    GpSimd (state-2 chain), Scalar/ACT (squares, sqrt, affine).

Self-contained: hardcodes shapes; no imports from /root/problem.
"""

import os
import sys
import time

sys.path.insert(0, "/opt/trn_rl_repo")

import numpy as np

import concourse.bass as bass
import concourse.mybir as mybir
import concourse.tile as tile
from concourse import bacc
from concourse import bass2jax
from concourse.bass_interp import get_hw_module

B, D, N = 256, 256, 16
DN = D * N                      # 4096 flattened (i,k) / (j,l)
T = int(os.environ.get("KUR_T", "25"))
GAMMA = 0.1
NCORES = 8
IK = DN // NCORES               # 512 ik per core (32 i values)
NI = D // NCORES                # 32 i per core
BT = 128                        # batches per group
NG = 2                          # batch groups
NCH = DN // 128                 # 32 contraction chunks

FP32 = mybir.dt.float32
FP16 = mybir.dt.float16

_CACHE = {}


def _build(nc):
    AF = mybir.ActivationFunctionType
    ALU = mybir.AluOpType

    # ---------------- DRAM I/O ----------------
    a_d = nc.dram_tensor("a_mat", [DN, IK], FP16, kind="ExternalInput")
    b_d = nc.dram_tensor("b_mat", [DN, IK], FP16, kind="ExternalInput")
    wbd_d = nc.dram_tensor("wbd", [128, IK], FP16, kind="ExternalInput")
    # per (group, kind): kind = [emb_s, emb_d, emb1, emb2] chunks, each [128,128]
    embs_d = nc.dram_tensor("embs", [128, NG * 4 * 128], FP16, kind="ExternalInput")
    ones_d = nc.dram_tensor("ones_r", [1, 128], FP16, kind="ExternalInput")
    bdr_d = nc.dram_tensor("bd_r", [1, IK], FP16, kind="ExternalInput")
    id_d = nc.dram_tensor("ident", [128, 128], FP16, kind="ExternalInput")
    noise_d = nc.dram_tensor("noise", [B, 2 * IK], FP32, kind="ExternalInput")
    out_d = nc.dram_tensor("xt_out", [B, 2 * IK], FP32, kind="ExternalOutput")

    # internal HBM: AG input (own transposed slice) + gathered state, x2 parity
    agin = [
        [nc.dram_tensor(f"agin{g}_{p}", [IK, 256], FP16) for p in range(2)]
        for g in range(NG)
    ]
    xg = [
        [
            nc.dram_tensor(f"xg{g}_{p}", [DN, 256], FP16, addr_space="Shared")
            for p in range(2)
        ]
        for g in range(NG)
    ]

    V = nc.vector
    P = nc.gpsimd
    S = nc.scalar

    with tile.TileContext(nc) as tc:
        with (
            tc.tile_pool(name="res", bufs=1) as res,
            tc.tile_pool(name="xgq", bufs=2) as xgq,
            tc.tile_pool(name="tmp", bufs=2) as tmp,
            tc.tile_pool(name="sd", bufs=2) as sdp,
            tc.tile_pool(name="fps", bufs=1, space="PSUM") as fps,
            tc.tile_pool(name="tps", bufs=1, space="PSUM") as tps,
        ):
            # ---------------- resident SBUF ----------------
            a_sb = res.tile([128, NCH * IK], FP16, tag="a")
            b_sb = res.tile([128, NCH * IK], FP16, tag="b")
            for k in range(NCH):
                nc.sync.dma_start(
                    out=a_sb[:, k * IK:(k + 1) * IK],
                    in_=a_d[k * 128:(k + 1) * 128, :],
                )
                nc.sync.dma_start(
                    out=b_sb[:, k * IK:(k + 1) * IK],
                    in_=b_d[k * 128:(k + 1) * 128, :],
                )
            wbd_sb = res.tile([128, IK], FP16, tag="wbd")
            nc.sync.dma_start(out=wbd_sb[:, :], in_=wbd_d[:, :])
            embs_sb = res.tile([128, NG * 4 * 128], FP16, tag="embs")
            nc.sync.dma_start(out=embs_sb[:, :], in_=embs_d[:, :])
            ones_sb = res.tile([1, 128], FP16, tag="ones")
            nc.sync.dma_start(out=ones_sb[:, :], in_=ones_d[:, :])
            bdr_sb = res.tile([1, IK], FP16, tag="bdr")
            nc.sync.dma_start(out=bdr_sb[:, :], in_=bdr_d[:, :])
            id_sb = res.tile([128, 128], FP16, tag="ident")
            nc.sync.dma_start(out=id_sb[:, :], in_=id_d[:, :])

            # local state x[g][st]: [128 b, 512 ik] fp32
            xs = [
                [
                    res.tile(
                        [128, IK], FP32, tag=f"x{g}_{st}", name=f"x{g}_{st}"
                    )
                    for st in range(2)
                ]
                for g in range(NG)
            ]

            prev_cc = [[None, None] for _ in range(NG)]
            prev_din = [[[], []] for _ in range(NG)]
            cur_q = [[None] * 4 for _ in range(NG)]

            def emb_chunk(g, kind):
                c0 = (g * 4 + kind) * 128
                return embs_sb[:, c0:c0 + 128]

            def win3(ap2d):
                """[128, 512] AP -> [128, 32, 16] windowed view."""
                return ap2d.rearrange("p (i k) -> p i k", k=16)

            def bcast(ap_small):
                """[128, 32] AP -> [128, 32, 16] broadcast view."""
                return ap_small[:, :, None].broadcast_to([128, NI, 16])

            def launch_gather(g, t):
                """s/d from x tiles -> PE transpose -> agin -> AllGather -> SBUF."""
                p = t % 2
                s_t = sdp.tile([128, IK], FP16, tag=f"s{g}")
                d_t = sdp.tile([128, IK], FP16, tag=f"d{g}")
                V.tensor_add(out=s_t[:, :], in0=xs[g][0][:, :], in1=xs[g][1][:, :])
                P.tensor_sub(out=d_t[:, :], in0=xs[g][0][:, :], in1=xs[g][1][:, :])
                tp = tps.tile([128, 8 * 128], FP16, tag=f"tp{g}")
                for m in range(4):
                    nc.tensor.transpose(
                        tp[:, (2 * m) * 128:(2 * m + 1) * 128],
                        s_t[:, m * 128:(m + 1) * 128],
                        id_sb[:, :],
                    )
                    nc.tensor.transpose(
                        tp[:, (2 * m + 1) * 128:(2 * m + 2) * 128],
                        d_t[:, m * 128:(m + 1) * 128],
                        id_sb[:, :],
                    )
                agst = sdp.tile([128, 8 * 128], FP16, tag=f"ag{g}", name=f"ag{g}")
                S.copy(agst[:, :], tp[:, :])
                ag_dmas = []
                for m in range(4):
                    dma = nc.sync.dma_start(
                        out=agin[g][p][m * 128:(m + 1) * 128, :],
                        in_=agst[:, 2 * m * 128:(2 * m + 2) * 128],
                    )
                    if prev_cc[g][p] is not None:
                        tile.add_dep_helper(
                            dma.ins, prev_cc[g][p].ins, reason="agin WAR vs prev AG"
                        )
                    ag_dmas.append(dma)
                cc = nc.gpsimd.collective_compute(
                    "AllGather",
                    ALU.bypass,
                    replica_groups=[list(range(NCORES))],
                    ins=[agin[g][p][:, :].opt()],
                    outs=[xg[g][p][:, :].opt()],
                )
                for dma in ag_dmas:
                    tile.add_dep_helper(cc.ins, dma.ins, reason="AG RAW on agin")
                for dma in prev_din[g][p]:
                    tile.add_dep_helper(cc.ins, dma.ins, reason="xg WAR vs prev read")
                prev_cc[g][p] = cc
                # gathered state -> SBUF quarters
                base = xg[g][p][:, :]
                dins = []
                for j in range(4):
                    tq = xgq.tile([128, 8 * 256], FP16, tag=f"xg{g}q{j}")
                    in_ap = bass.AP(
                        tensor=base.tensor,
                        offset=base.offset + j * 1024 * 256,
                        ap=[[256, 128], [128 * 256, 8], [1, 256]],
                    )
                    dma = nc.sync.dma_start(out=tq[:, :], in_=in_ap)
                    tile.add_dep_helper(dma.ins, cc.ins, reason="stream RAW on AG")
                    dins.append(dma)
                    cur_q[g][j] = tq
                prev_din[g][p] = dins

            def elementwise(g, u, v):
                """u,v PSUM [128,512] fp32 -> updated x tiles (in-place)."""
                x1, x2 = xs[g][0], xs[g][1]
                vs = tmp.tile([128, IK], FP32, tag="vs")
                S.copy(vs[:, :], v[:, :])
                h1 = tmp.tile([128, IK], FP32, tag="h1")
                h2 = tmp.tile([128, IK], FP32, tag="h2")
                V.tensor_add(out=h1[:, :], in0=u[:, :], in1=vs[:, :])
                V.tensor_sub(out=h2[:, :], in0=u[:, :], in1=vs[:, :])
                xf1 = tmp.tile([128, IK], FP32, tag="xf1")
                xf2 = tmp.tile([128, IK], FP32, tag="xf2")
                P.tensor_mul(out=xf1[:, :], in0=x1[:, :], in1=h1[:, :])
                P.tensor_mul(out=xf2[:, :], in0=x2[:, :], in1=h2[:, :])
                dot1 = tmp.tile([128, NI], FP32, tag="dot1")
                dot2 = tmp.tile([128, NI], FP32, tag="dot2")
                V.tensor_reduce(
                    out=dot1[:, :], in_=win3(xf1[:, :]),
                    axis=mybir.AxisListType.X, op=ALU.add,
                )
                V.tensor_reduce(
                    out=dot2[:, :], in_=win3(xf2[:, :]),
                    axis=mybir.AxisListType.X, op=ALU.add,
                )
                g1 = tmp.tile([128, NI], FP32, tag="g1")
                g2 = tmp.tile([128, NI], FP32, tag="g2")
                # g = 1 - gamma*dot
                S.activation(g1[:, :], dot1[:, :], AF.Copy, bias=1.0, scale=-GAMMA)
                S.activation(g2[:, :], dot2[:, :], AF.Copy, bias=1.0, scale=-GAMMA)
                xg1 = tmp.tile([128, IK], FP32, tag="xg1")
                xg2 = tmp.tile([128, IK], FP32, tag="xg2")
                P.tensor_mul(out=win3(xg1[:, :]), in0=win3(x1[:, :]), in1=bcast(g1))
                P.tensor_mul(out=win3(xg2[:, :]), in0=win3(x2[:, :]), in1=bcast(g2))
                pre1 = tmp.tile([128, IK], FP32, tag="pre1")
                pre2 = tmp.tile([128, IK], FP32, tag="pre2")
                V.scalar_tensor_tensor(
                    out=pre1[:, :], in0=h1[:, :], scalar=GAMMA, in1=xg1[:, :],
                    op0=ALU.mult, op1=ALU.add,
                )
                V.scalar_tensor_tensor(
                    out=pre2[:, :], in0=h2[:, :], scalar=GAMMA, in1=xg2[:, :],
                    op0=ALU.mult, op1=ALU.add,
                )
                _norm_apply(g, pre1, pre2)

            def _norm_apply(g, pre1, pre2):
                """x[g][st] = pre_st / ||pre_st|| (windowed over 16)."""
                x1, x2 = xs[g][0], xs[g][1]
                sq1 = tmp.tile([128, IK], FP32, tag="sq1")
                sq2 = tmp.tile([128, IK], FP32, tag="sq2")
                S.square(sq1[:, :], pre1[:, :])
                S.square(sq2[:, :], pre2[:, :])
                n21 = tmp.tile([128, NI], FP32, tag="n21")
                n22 = tmp.tile([128, NI], FP32, tag="n22")
                V.tensor_reduce(
                    out=n21[:, :], in_=win3(sq1[:, :]),
                    axis=mybir.AxisListType.X, op=ALU.add,
                )
                V.tensor_reduce(
                    out=n22[:, :], in_=win3(sq2[:, :]),
                    axis=mybir.AxisListType.X, op=ALU.add,
                )
                nrm1 = tmp.tile([128, NI], FP32, tag="nrm1")
                nrm2 = tmp.tile([128, NI], FP32, tag="nrm2")
                S.sqrt(nrm1[:, :], n21[:, :])
                S.sqrt(nrm2[:, :], n22[:, :])
                rv1 = tmp.tile([128, NI], FP32, tag="rv1")
                rv2 = tmp.tile([128, NI], FP32, tag="rv2")
                V.reciprocal(out=rv1[:, :], in_=nrm1[:, :])
                V.reciprocal(out=rv2[:, :], in_=nrm2[:, :])
                V.tensor_mul(out=win3(x1[:, :]), in0=win3(pre1[:, :]), in1=bcast(rv1))
                P.tensor_mul(out=win3(x2[:, :]), in0=win3(pre2[:, :]), in1=bcast(rv2))

            # ---------------- init: x0 = normalize(noise + c) ----------------
            for g in range(NG):
                for st in range(2):
                    cps = fps.tile([128, IK], FP32, tag=f"u{g}")
                    nc.tensor.matmul(
                        cps[:, :], ones_sb[:, :], bdr_sb[:, :],
                        start=True, stop=False, skip_group_check=True,
                    )
                    nc.tensor.matmul(
                        cps[:, :], emb_chunk(g, 2 + st), wbd_sb[:, :],
                        start=False, stop=True, skip_group_check=True,
                    )
                    nt = tmp.tile([128, IK], FP32, tag="noise")
                    nc.sync.dma_start(
                        out=nt[:, :],
                        in_=noise_d[g * 128:(g + 1) * 128, st * IK:(st + 1) * IK],
                    )
                    pre = tmp.tile([128, IK], FP32, tag=f"pre{st + 1}")
                    V.tensor_add(out=pre[:, :], in0=cps[:, :], in1=nt[:, :])
                    if st == 0:
                        pre1 = pre
                    else:
                        _norm_apply(g, pre1, pre)
                launch_gather(g, 0)

            # ---------------- main loop ----------------
            # The PE queue is in-order: group g's transposes (which wait on
            # g's elementwise) are emitted only after the OTHER group's MM
            # phase so they never block it.
            pending_tail = None
            for t in range(T):
                for g in range(NG):
                    u = fps.tile([128, IK], FP32, tag=f"u{g}")
                    v = fps.tile([128, IK], FP32, tag=f"v{g}")
                    # c-folds: u += b_d + emb_s (x) W ; v += emb_d (x) W
                    nc.tensor.matmul(
                        u[:, :], ones_sb[:, :], bdr_sb[:, :],
                        start=True, stop=False, skip_group_check=True,
                    )
                    nc.tensor.matmul(
                        u[:, :], emb_chunk(g, 0), wbd_sb[:, :],
                        start=False, stop=False, skip_group_check=True,
                    )
                    nc.tensor.matmul(
                        v[:, :], emb_chunk(g, 1), wbd_sb[:, :],
                        start=True, stop=False, skip_group_check=True,
                    )
                    for k in range(NCH):
                        if k == 16 and pending_tail is not None:
                            # other group's transpose+AllGather tail, emitted
                            # mid-phase so the AG flies under our remaining
                            # chunks (its PE transposes only wait on the other
                            # group's elementwise, which overlapped chunks
                            # 0..15).
                            launch_gather(*pending_tail)
                            pending_tail = None
                        tq = cur_q[g][k // 8]
                        c0 = (k % 8) * 256
                        s_chunk = tq[:, c0:c0 + 128]
                        d_chunk = tq[:, c0 + 128:c0 + 256]
                        last = k == NCH - 1
                        nc.tensor.matmul(
                            u[:, :], s_chunk, a_sb[:, k * IK:(k + 1) * IK],
                            start=False, stop=last, skip_group_check=True,
                        )
                        nc.tensor.matmul(
                            v[:, :], d_chunk, b_sb[:, k * IK:(k + 1) * IK],
                            start=False, stop=last, skip_group_check=True,
                        )
                    elementwise(g, u, v)
                    if t < T - 1:
                        pending_tail = (g, t + 1)

            # ---------------- output ----------------
            for g in range(NG):
                for st in range(2):
                    nc.sync.dma_start(
                        out=out_d[g * 128:(g + 1) * 128, st * IK:(st + 1) * IK],
                        in_=xs[g][st][:, :],
                    )

    nc.compile()
    return nc


def _get_nc():
    if "nc" not in _CACHE:
        nc = bacc.Bacc(
            "TRN2", target_bir_lowering=False, debug=False, num_devices=NCORES
        )
        _build(nc)
        nc.m = get_hw_module(nc.m)
        _CACHE["nc"] = nc
    return _CACHE["nc"]


def _marshal(embeddings1, embeddings2, W_d, b_d, J_in, J_out, Omega, noise1, noise2):
    """Host-side pure data movement + linear prep: build A/Bm slices etc."""
    f32 = np.float32
    AT = (J_in + J_out).transpose(1, 3, 0, 2).reshape(DN, DN).astype(f32) * 0.5
    BT2 = (J_in - J_out).transpose(1, 3, 0, 2).reshape(DN, DN).astype(f32) * 0.5
    for i in range(D):
        blk = 0.5 * Omega[i].T  # [l,k]
        AT[i * N:(i + 1) * N, i * N:(i + 1) * N] += blk
        BT2[i * N:(i + 1) * N, i * N:(i + 1) * N] += blk
    emb_s = 0.5 * (embeddings1 + embeddings2)
    emb_d = 0.5 * (embeddings1 - embeddings2)
    n1 = noise1.reshape(B, DN)
    n2 = noise2.reshape(B, DN)
    bd_flat = b_d.reshape(DN)

    in_maps = []
    for q in range(NCORES):
        ik0 = IK * q
        i0 = NI * q
        j0 = q // 4
        wbd = np.zeros((128, IK), f32)
        r0 = i0 - 128 * j0
        for il in range(NI):
            wbd[r0 + il, il * N:(il + 1) * N] = W_d[i0 + il]
        embs = np.zeros((128, NG * 4 * 128), f32)
        for g in range(NG):
            bsl = slice(128 * g, 128 * (g + 1))
            for kind, e in enumerate((emb_s, emb_d, embeddings1, embeddings2)):
                embs[:, (g * 4 + kind) * 128:(g * 4 + kind + 1) * 128] = (
                    e[bsl, 128 * j0:128 * (j0 + 1)].T
                )
        noise = np.concatenate(
            [n1[:, ik0:ik0 + IK], n2[:, ik0:ik0 + IK]], axis=1
        )
        in_maps.append(
            {
                "a_mat": np.ascontiguousarray(AT[:, ik0:ik0 + IK]).astype(np.float16),
                "b_mat": np.ascontiguousarray(BT2[:, ik0:ik0 + IK]).astype(np.float16),
                "wbd": wbd.astype(np.float16),
                "embs": embs.astype(np.float16),
                "ones_r": np.ones((1, 128), np.float16),
                "bd_r": bd_flat[ik0:ik0 + IK][None].astype(np.float16),
                "ident": np.eye(128, dtype=np.float16),
                "noise": np.ascontiguousarray(noise, f32),
            }
        )
    return in_maps


def _unmarshal(results):
    out = np.empty((2, B, D, N), np.float32)
    for q in range(NCORES):
        xt = results[q]["xt_out"]  # [256, 1024]
        i0 = NI * q
        out[0][:, i0:i0 + NI, :] = xt[:, :IK].reshape(B, NI, N)
        out[1][:, i0:i0 + NI, :] = xt[:, IK:].reshape(B, NI, N)
    return out


def run_on_device(in_maps):
    nc = _get_nc()
    return bass2jax.run_bass_via_pjrt(nc, in_maps, n_cores=NCORES)


def kernel(**inputs):
    in_maps = _marshal(**{k: np.asarray(v, np.float32) for k, v in inputs.items()})
    results = run_on_device(in_maps)
    return _unmarshal(results)


if __name__ == "__main__":
    rng = np.random.default_rng(0)
    ins = {
        "embeddings1": rng.standard_normal((B, D), dtype=np.float32),
        "embeddings2": rng.standard_normal((B, D), dtype=np.float32),
        "W_d": rng.standard_normal((D, N), dtype=np.float32) * 0.1,
        "b_d": np.zeros((D, N), np.float32),
        "J_in": (rng.standard_normal((D, D, N, N), dtype=np.float32) * 0.007),
        "J_out": (rng.standard_normal((D, D, N, N), dtype=np.float32) * 0.007),
        "Omega": rng.standard_normal((D, N, N), dtype=np.float32) * 0.1,
        "noise1": rng.standard_normal((B, D, N), dtype=np.float32) * 0.05,
        "noise2": rng.standard_normal((B, D, N), dtype=np.float32) * 0.05,
    }
    t0 = time.time()
    out = kernel(**ins)
    print("kernel() took", time.time() - t0, "s; out shape", out.shape)
